# revision 38
# baseline (speedup 1.0000x reference)
"""Trainium2 Bass kernel for nn_ODEFunc_90159953478502 (MoE routing, inference path).

Math (see reference):
    logits  = x @ Wg[:256] + (t*Wg[512] + bg)      # zeros kill Wg[256:512]
    w       = softmax(logits, axis=-1)             # [B, E]
    eo_e    = tanh(x @ W1[e] + b1[e]) @ W2[e] + b2[e]
    active_e = any_b(w[b,e] > 0.01)
    out     = sum_e active_e * w[:,e,None] * eo_e  # >=1 active always:
                                                   # max softmax >= 1/8 > 0.01

Sharding: expert-parallel. Core e holds the full batch plus only W1[e]/W2[e],
computes w[:,e,None] * (tanh(x@W1[e]+b1[e]) @ W2[e]) in transposed layout
([D, B]) plus a 1-element activity mask m_e = any(w[:,e] > 0.01); the host
sums m_e-masked partial outputs. The b2 rank-1 term (zero here) is added
host-side from a numpy gating replay only when b2 != 0.

Device structure per core (all matmul IO fp16, 1 cycle/row on PE + FWL;
fp32 was ~280ns per [128,128]@[128,512] vs ~215ns fp16):
  - all inputs packed host-side into 5 tensors / 9 total dma_starts.
    Each HWDGE dma_start costs ~700ns serialized on the Sync queue, so
    the old 100+ small DMAs (~82us of trigger time) collapse to ~6us.
  - a few warm-up matmuls on a memset tile ramp the PE HAM clock
    (1.2->2.4GHz) while input DMAs are in flight.
  - gating interleaved per chunk c (B split into 8 chunks of 512):
    logits^T [8,512] via lhsT=wgx (2 d-tiles), ACT Exp with fused +gbias
    (no max-subtract: |logits| <= ~6), then ONE [8,2] sel matmul gives
    S (ones column) and E_e (onehot column) rows; w_e = E_e * recip(S).
    The w row is DRAM-bounced and partition-broadcast to wb [128,512]
    (SWDGE on the GpSimd queue, off the critical path).
  - mask: running max of w_e across chunks, one compare vs 0.01 + reduce
    at the end -> MASK [1,1] output. No mask work on the main path.
  - main pipeline per chunk: mm1 (tanh( x@W1 )) into fp16 ht tiles, mm2
    (@W2) one chunk behind; drain = single DVE mult by wb, fp16 output,
    16 output stores [128,512].
"""

import sys

if "/opt/trn_rl_repo" not in sys.path:
    sys.path.insert(0, "/opt/trn_rl_repo")

import numpy as np

_B, _D, _H, _E = 4096, 256, 1024, 8
_NCORES = 8
_CHUNK = 512
_NCH = _B // _CHUNK
_DT = _D // 128   # 2 d-tiles
_HT = _H // 128   # 8 h-tiles
_THRESH = 0.01
# PE warm-up matmuls while input DMAs land: enough to keep the PE busy
# from engine start (~7.8us) until x chunk 0 arrives (~10.5us) so the HAM
# clock governor flips to 2.4GHz at ~11.2us instead of ~23us (a ~3.4us
# idle or cold window keeps everything at 1.2GHz)
_NWARM = 6

_CACHE = {}


def _build():
    import concourse.bass as bass
    import concourse.tile as tile
    import concourse.mybir as mybir
    from concourse import bacc
    from contextlib import ExitStack

    F32 = mybir.dt.float32
    F16 = mybir.dt.float16
    AF = mybir.ActivationFunctionType
    ALU = mybir.AluOpType
    AX = mybir.AxisListType

    nc = bacc.Bacc("TRN2", target_bir_lowering=False, debug=False)

    # packed inputs (see _make_in_maps for layouts). Every dma_start's DRAM
    # side is one contiguous block — strided per-partition lines (2KB each)
    # run at less than half DMA bandwidth (descriptor-dominated)
    XP = nc.declare_dram_parameter("XP", [_NCH * 128, _DT * _CHUNK], F16, isOutput=False)
    W1P = nc.declare_dram_parameter("W1P", [_DT * 128, _H], F16, isOutput=False)
    W2P = nc.declare_dram_parameter("W2P", [_DT * 128, _H], F16, isOutput=False)
    # gating weights padded to 128 cols per d-tile: FWL (hidden LDWEIGHTS)
    # needs a full 128-column lhsT
    CH = nc.declare_dram_parameter("CH", [128, _DT * 128 + 2], F16, isOutput=False)
    CF = nc.declare_dram_parameter("CF", [128, 1 + _HT], F32, isOutput=False)
    OUTP = nc.declare_dram_parameter("OUTP", [_DT * _NCH * 128, _CHUNK], F16, isOutput=True)
    MASK = nc.declare_dram_parameter("MASK", [1, 1], F32, isOutput=True)

    def bcast(src_ap, n):
        # [1, L] view -> [n, L] partition-broadcast view (stride-0 partitions)
        step, cnt = src_ap.ap[-1]
        return bass.AP(tensor=src_ap.tensor, offset=src_ap.offset, ap=[[0, n], [step, cnt]])

    with tile.TileContext(nc) as tc, ExitStack() as ctx:
        const = ctx.enter_context(tc.tile_pool(name="const", bufs=1))
        dpool = ctx.enter_context(tc.tile_pool(name="dram", bufs=1, space="DRAM"))
        small = ctx.enter_context(tc.tile_pool(name="small", bufs=4))
        wrp = ctx.enter_context(tc.tile_pool(name="wrp", bufs=4))
        wbp = ctx.enter_context(tc.tile_pool(name="wbp", bufs=4))
        htp = ctx.enter_context(tc.tile_pool(name="htp", bufs=18))
        op = ctx.enter_context(tc.tile_pool(name="op", bufs=5))
        pg = ctx.enter_context(tc.tile_pool(name="pg", bufs=2, space="PSUM"))
        ph = ctx.enter_context(tc.tile_pool(name="ph", bufs=4, space="PSUM"))
        po = ctx.enter_context(tc.tile_pool(name="po", bufs=2, space="PSUM"))

        # ---- PE warm-up: memset a junk tile, run matmuls on it ------------
        warm = const.tile([128, _CHUNK], F16)
        nc.gpsimd.memset(warm[:], 0.0)
        for _ in range(_NWARM):
            psw = ph.tile([128, _CHUNK], F32, tag="psh")
            nc.tensor.matmul(psw[:], warm[:, 0:128], warm[:], start=True, stop=True)

        # ---- inputs: few big DMAs, ordered by first use -------------------
        # x chunk 0 + consts go through the GpSimd SWDGE queue, which starts
        # issuing ~1us before the Sync queue — gating(0) can start ~10.5us.
        # W1 (split in two for earlier mm1(0)) and the rest go via Sync.
        ch_sb = const.tile([128, _DT * 128 + 2], F16)
        cf_sb = const.tile([128, 1 + _HT], F32)
        xp = const.tile([128, _NCH * _DT * _CHUNK], F16)
        w1 = const.tile([128, _DT * _H], F16)
        w2 = const.tile([128, _HT * _D], F16)

        def xload(c, eng):
            eng.dma_start(
                xp[:, c * _DT * _CHUNK : (c + 1) * _DT * _CHUNK],
                XP.ap()[c * 128 : (c + 1) * 128, :],
            )

        # ~0.25MB contiguous slices in exact consumption order: the 16 SDMA
        # engines round-robin across queues, so anything queued early steals
        # bandwidth from the load the PE is actually waiting on
        xload(0, nc.gpsimd)
        nc.gpsimd.dma_start(ch_sb[:], CH.ap())
        nc.gpsimd.dma_start(cf_sb[:], CF.ap())
        nc.sync.dma_start(w1[:, : _H], W1P.ap()[0:128, :])
        nc.sync.dma_start(w1[:, _H :], W1P.ap()[128:256, :])
        xload(1, nc.sync)
        xload(2, nc.sync)
        nc.sync.dma_start(w2[:, : _H], W2P.ap()[0:128, :])
        xload(3, nc.sync)
        nc.sync.dma_start(w2[:, _H :], W2P.ap()[128:256, :])
        for c in range(4, 8):
            xload(c, nc.sync)

        def xm(d, c):  # [128, 512] fp16 rhs view of x chunk c, d-tile d
            o = c * _DT * _CHUNK + d * _CHUNK
            return xp[:, o : o + _CHUNK]

        def w1t(d, hh):  # [128, 128] lhsT view
            o = d * _H + hh * 128
            return w1[:, o : o + 128]

        def w2t(hh, d2):  # [128, 128] lhsT view (W2P packed d2-major)
            o = d2 * _H + hh * 128
            return w2[:, o : o + 128]

        wgx = lambda d: ch_sb[:, d * 128 : d * 128 + 128]    # [128, 128] padded
        sel = ch_sb[0:8, _DT * 128 : _DT * 128 + 2]          # [8, 2]
        gb = cf_sb[0:8, 0:1]                                 # [8, 1]
        b1c = lambda hh: cf_sb[:, 1 + hh : 2 + hh]           # [128, 1]

        wrow_d = dpool.tile([1, _B], F32)
        rmax = small.tile([1, _CHUNK], F32)

        wb_tiles = {}
        ht_by_chunk = {}

        def gating(c):
            cs = slice(c * _CHUNK, (c + 1) * _CHUNK)
            # M=128 (rows 8..127 are zero padding) so FWL hides LDWEIGHTS
            psg = pg.tile([128, _CHUNK], F32, tag="pg")
            for d in range(_DT):
                nc.tensor.matmul(psg[:], wgx(d), xm(d, c), start=(d == 0), stop=(d == _DT - 1))
            e_sb = small.tile([_E, _CHUNK], F16, tag="e_sb")
            nc.scalar.activation(e_sb[:], psg[0:_E, :], AF.Exp, bias=gb)
            # S at PSUM partition 0, E_e at partition 32: distinct PE
            # column-groups via tile_position, so both matmuls run
            # concurrently (~one matmul time) and both rows are readable
            # (PSUM/SBUF access must start at a 32-aligned partition)
            pss = pg.tile([33, _CHUNK], F32, tag="pg")
            nc.tensor.matmul(pss[0:1, :], sel[:, 0:1], e_sb[:], start=True, stop=True, tile_position=(0, 0))
            nc.tensor.matmul(pss[32:33, :], sel[:, 1:2], e_sb[:], start=True, stop=True, tile_position=(0, 32))
            recip = small.tile([1, _CHUNK], F32, tag="recip")
            nc.vector.reciprocal_approx_fast(recip[:], pss[0:1, :])
            wu = wrp.tile([1, _CHUNK], F32, tag="wu")
            nc.vector.tensor_tensor(wu[:], pss[32:33, :], recip[:], ALU.mult)
            if c == 0:
                nc.vector.tensor_copy(rmax[:], wu[:])
            else:
                nc.vector.tensor_tensor(rmax[:], rmax[:], wu[:], ALU.max)
            nc.gpsimd.dma_start(wrow_d[0:1, cs], wu[:])
            wb = wbp.tile([128, _CHUNK], F32, tag="wb")
            nc.gpsimd.dma_start(wb[:], bcast(wrow_d[0:1, cs], 128))
            wb_tiles[c] = wb

        def mm1(c):
            ht_tiles = []
            for hh in range(_HT):
                psh = ph.tile([128, _CHUNK], F32, tag="psh")
                for d in range(_DT):
                    nc.tensor.matmul(psh[:], w1t(d, hh), xm(d, c), start=(d == 0), stop=(d == _DT - 1))
                ht = htp.tile([128, _CHUNK], F16, tag="ht")
                nc.scalar.activation(ht[:], psh[:], AF.Tanh, bias=b1c(hh))
                ht_tiles.append(ht)
            ht_by_chunk[c] = ht_tiles

        def mm2(c):
            ht_tiles = ht_by_chunk.pop(c)
            last = c == _NCH - 1
            for d2 in range(_DT):
                pso = po.tile([128, _CHUNK], F32, tag="pso")
                for hh in range(_HT):
                    nc.tensor.matmul(pso[:], w2t(hh, d2), ht_tiles[hh][:], start=(hh == 0), stop=(hh == _HT - 1))
                osb = op.tile([128, _CHUNK], F16, tag="osb")
                r = (d2 * _NCH + c) * 128
                if last and d2 == _DT - 1:
                    # final tile: drain+store in halves so the first store's
                    # completion overlaps the second half's drain
                    for q in range(2):
                        hs = slice(q * (_CHUNK // 2), (q + 1) * (_CHUNK // 2))
                        nc.vector.tensor_tensor(osb[:, hs], pso[:, hs], wb_tiles[c][:, hs], ALU.mult)
                        nc.sync.dma_start(OUTP.ap()[r : r + 128, hs], osb[:, hs])
                else:
                    nc.vector.tensor_tensor(osb[:], pso[:], wb_tiles[c][:], ALU.mult)
                    nc.sync.dma_start(OUTP.ap()[r : r + 128, :], osb[:])

        def mask_out():
            # active mask from the running row max; issued right after the
            # last gating chunk so its store isn't serialized into the tail
            mtmp = small.tile([1, _CHUNK], F32, tag="recip")
            nc.vector.tensor_scalar(out=mtmp[:], in0=rmax[:], scalar1=_THRESH, scalar2=None, op0=ALU.is_gt)
            m_sb = small.tile([1, 1], F32, tag="m_sb")
            nc.vector.reduce_max(m_sb[:], mtmp[:], axis=AX.X)
            nc.sync.dma_start(MASK.ap(), m_sb[:])

        for c in range(_NCH):
            gating(c)
            if c == _NCH - 1:
                mask_out()
            mm1(c)
            if c >= 1:
                mm2(c - 1)
        mm2(_NCH - 1)

    nc.finalize()
    return nc


def _get_nc():
    if "nc" not in _CACHE:
        _CACHE["nc"] = _build()
    return _CACHE["nc"]


def _make_in_maps(t, x, W1, b1, W2, b2, Wg, bg):
    f16 = np.float16
    xT = np.ascontiguousarray(x.T).astype(np.float32, copy=False)
    # x packed chunk-major rows: XP[c*128+p, d*512+b] = xT[d*128+p, c*512+b]
    xP = np.ascontiguousarray(
        xT.reshape(_DT, 128, _NCH, _CHUNK).transpose(2, 1, 0, 3).reshape(_NCH * 128, -1)
    ).astype(f16)
    wgxP = (
        np.asarray(Wg[: _D], dtype=np.float32)
        .reshape(_DT, 128, _E)
        .transpose(1, 0, 2)
    )  # [128, _DT, _E]
    gb = (np.float32(t[0]) * Wg[2 * _D] + bg).astype(np.float32).reshape(_E, 1)
    in_maps = []
    for c in range(_NCORES):
        # CH fp16: [wgx d0 (128 cols, 8 used) | wgx d1 (128 cols) | sel (2)]
        chP = np.zeros((128, _DT * 128 + 2), dtype=f16)
        for d in range(_DT):
            chP[:, d * 128 : d * 128 + _E] = wgxP[:, d, :].astype(f16)
        chP[0:8, _DT * 128] = 1.0       # ones column -> S
        chP[c, _DT * 128 + 1] = 1.0     # onehot column -> E_e
        # CF fp32: [gb (1 col) | b1 tiles (8 cols)]
        cfP = np.zeros((128, 1 + _HT), dtype=np.float32)
        cfP[0:8, 0:1] = gb
        cfP[:, 1:] = np.asarray(b1[c], dtype=np.float32).reshape(_HT, 128).T
        in_maps.append(
            {
                "XP": xP,
                # natural layout: W1P[d*128+p, h] = W1[d*128+p, h]
                "W1P": np.ascontiguousarray(np.asarray(W1[c])).astype(f16),
                # d2-major rows: W2P[d2*128+r, hh*128+q] = W2[hh*128+r, d2*128+q]
                "W2P": np.ascontiguousarray(
                    np.asarray(W2[c], dtype=np.float32)
                    .reshape(_HT, 128, _DT, 128)
                    .transpose(2, 1, 0, 3)
                    .reshape(_DT * 128, _H)
                ).astype(f16),
                "CH": chP,
                "CF": cfP,
            }
        )
    return in_maps


def _assemble(results, inputs):
    out = np.zeros((_B, _D), dtype=np.float64)
    masks = []
    for c in range(_NCORES):
        m = float(results[c]["MASK"][0, 0]) > 0.5
        masks.append(m)
        if m:
            # OUTP[(d*8+cc)*128+p, s] -> out[cc*512+s, d*128+p]
            o = results[c]["OUTP"].astype(np.float64).reshape(_DT, _NCH, 128, _CHUNK)
            out += o.transpose(1, 3, 0, 2).reshape(_B, _D)
    # all-inactive fallback is unreachable: softmax max >= 1/E = 0.125 > 0.01
    b2 = np.asarray(inputs["b2"])
    if np.any(b2):
        # rank-1 bias term sum_e m_e * w[:,e] b2[e,:] — numpy gating replay
        t, x, Wg, bg = (np.asarray(inputs[k]) for k in ("t", "x", "Wg", "bg"))
        logits = x.astype(np.float64) @ Wg[:_D].astype(np.float64)
        logits += np.float64(t[0]) * Wg[2 * _D].astype(np.float64) + bg
        ex = np.exp(logits - logits.max(axis=1, keepdims=True))
        w = ex / ex.sum(axis=1, keepdims=True)
        active = (w > _THRESH).any(axis=0)
        out += (w * active) @ b2.astype(np.float64)
    return out.astype(np.float32)


def run_on_device(t, x, W1, b1, W2, b2, Wg, bg, trace=False):
    from concourse.bass_utils import run_bass_kernel_spmd

    inputs = dict(t=t, x=x, W1=W1, b1=b1, W2=W2, b2=b2, Wg=Wg, bg=bg)
    in_maps = _make_in_maps(**inputs)
    res = run_bass_kernel_spmd(
        _get_nc(), in_maps, list(range(_NCORES)), trace=trace
    )
    return _assemble(res.results, inputs), res


def kernel(t, x, W1, b1, W2, b2, Wg, bg):
    out, _ = run_on_device(t, x, W1, b1, W2, b2, Wg, bg, trace=False)
    return out
